# revision 13
# baseline (speedup 1.0000x reference)
"""Trainium2 Bass kernel: fused bmm+decay+reduce attention scorer.

Computes, for full inputs
    self_attn  [N=16, M=100, EMB=128] f32
    self_delta [N=16, M=100, L=10000, D=4] f32
    emb_table  [L+1=10001, EMB=128] f32
    value_w    [M=100] f32
the output
    out[n, l] = sum_m value_w[m] * (sum_d self_delta[n,m,l,d]) * (emb_table[1+l] . self_attn[n,m])
of shape [16, 10000] f32 (matches the reference jnp einsum chain).

Sharding: the candidate/location axis L is split 8 ways (1250 locations per
core); every core handles all 16 batch rows for its location range.  This
keeps the dominant stream (self_delta, 32 MB/core) un-replicated and only
replicates the small attn/value tensors; the embedding table is row-sharded.

Layout: the 16*100 (batch, step) rows are flattened to r = n*100 + m and
processed in 13 partition-major tiles of 128 rows.  The delta blob is staged
host-side as [1600, LSH*4] row-major, so every DMA is a dense [128 x 20KB]
transfer whose descriptors read contiguous HBM - the fast path for the 16
SDMA engines.  A vw-scaled one-hot stationary operand vwoh[p, j] =
vw[m(r)] * (n(r) == j) routes each partition's contribution to its own
output row during the m-contraction, so mixing different n in one tile is
fine.  Per tile: PE computes S[p,l] = emb[l].attn[r(p)] (fp32r single-pass
matmul), DVE reduces D with two pairwise adds and multiplies by S, and the
one-hot fp32r matmul accumulates all 16 output rows in PSUM across tiles.
"""

import numpy as np

import concourse.mybir as mybir
import concourse.tile as tile
from concourse import bacc
from concourse.bass_utils import run_bass_kernel_spmd

N, M, L, EMB, D = 16, 100, 10000, 128, 4
NCORES = 8
LSH = L // NCORES  # 1250 locations per core
R = N * M  # 1600 flattened (n, m) rows
P = 128
NTILE = (R + P - 1) // P  # 13 tiles; last holds 64 rows
# matmul moving-operand chunks: all >=256 (fp32r full-rate), <=512 (PSUM bank),
# and even-sized at even offsets (fp32r ISA restriction)
CHUNKS = [(0, 418), (418, 416), (834, 416)]
NCHUNK = len(CHUNKS)
FP32 = mybir.dt.float32
FP32R = mybir.dt.float32r

_NC_CACHE = {}


def _build_nc():
    nc = bacc.Bacc(
        "TRN2", target_bir_lowering=False, debug=False, num_devices=NCORES
    )
    raw_d = nc.dram_tensor("raw", [R, LSH * D], FP32, kind="ExternalInput").ap()
    embT_d = nc.dram_tensor("embT", [EMB, LSH], FP32R, kind="ExternalInput").ap()
    attnT_d = nc.dram_tensor(
        "attnT", [EMB, NTILE * P], FP32R, kind="ExternalInput"
    ).ap()
    vwoh_d = nc.dram_tensor("vwoh", [P, NTILE * N], FP32R, kind="ExternalInput").ap()
    out_d = nc.dram_tensor("out", [N, LSH], FP32, kind="ExternalOutput").ap()

    with tile.TileContext(nc) as tc:
        with (
            tc.tile_pool(name="const", bufs=1) as cpool,
            tc.tile_pool(name="raws", bufs=6) as rpool,
            tc.tile_pool(name="work", bufs=2) as wpool,
            tc.tile_pool(name="ppool", bufs=2) as ppool,
            tc.tile_pool(name="spsum", bufs=4, space="PSUM") as spool,
            tc.tile_pool(name="apsum", bufs=1, space="PSUM") as apool,
        ):
            embT = cpool.tile([EMB, LSH], FP32R, tag="embT")
            nc.scalar.dma_start(out=embT, in_=embT_d)
            attnT = cpool.tile([EMB, NTILE * P], FP32R, tag="attnT")
            nc.scalar.dma_start(out=attnT, in_=attnT_d)
            vwoh = cpool.tile([P, NTILE * N], FP32R, tag="vwoh")
            nc.scalar.dma_start(out=vwoh, in_=vwoh_d)

            # out accumulator rows n=0..15, one PSUM bank per l-chunk
            acc = apool.tile([N, NCHUNK, 512], FP32, tag="acc")

            for t in range(NTILE):
                rows = min(P, R - t * P)  # 128, or 64 in the last tile
                raw = rpool.tile([P, LSH * D], FP32, tag="raw")
                nc.sync.dma_start(
                    out=raw[:rows], in_=raw_d[t * P : t * P + rows]
                )
                rv = raw.rearrange("p (l d) -> p l d", d=D)

                # delta[p,l] = sum_d raw[p,l,d] via pairwise adds
                # (garbage rows beyond `rows` are masked by zero vwoh columns)
                a1 = wpool.tile([P, LSH, 2], FP32, tag="a1")
                nc.vector.tensor_add(out=a1, in0=rv[:, :, 0:2], in1=rv[:, :, 2:4])
                a2 = wpool.tile([P, LSH], FP32, tag="a2")
                nc.vector.tensor_add(out=a2, in0=a1[:, :, 0], in1=a1[:, :, 1])

                # S[p, l] = sum_k attn[r(p)] . emb_cand[lo+l]
                # one rotating single-bank PSUM tile per chunk, so chunk j's
                # multiply can release its bank while chunk j+1 still computes
                s_tiles = []
                for j, (c0, w) in enumerate(CHUNKS):
                    s = spool.tile([P, 512], FP32, tag="s")
                    nc.tensor.matmul(
                        s[:, :w],
                        attnT[:, t * P : (t + 1) * P],
                        embT[:, c0 : c0 + w],
                        start=True,
                        stop=True,
                    )
                    s_tiles.append(s)

                # Pt[p, l] = delta[p, l] * S[p, l], per bank-aligned chunk
                p_t = ppool.tile([P, LSH], FP32R, tag="p")
                for j, (c0, w) in enumerate(CHUNKS):
                    nc.vector.tensor_mul(
                        out=p_t[:, c0 : c0 + w],
                        in0=a2[:, c0 : c0 + w],
                        in1=s_tiles[j][:, :w],
                    )

                # acc[n, l] += sum_p vw[m(p)] * (n(p)==n) * Pt[p, l]
                for j, (c0, w) in enumerate(CHUNKS):
                    nc.tensor.matmul(
                        acc[:, j, :w],
                        vwoh[:, t * N : (t + 1) * N],
                        p_t[:, c0 : c0 + w],
                        start=(t == 0),
                        stop=(t == NTILE - 1),
                    )

            out_sb = cpool.tile([N, LSH], FP32, tag="out_sb")
            for j, (c0, w) in enumerate(CHUNKS):
                nc.any.tensor_copy(out=out_sb[:, c0 : c0 + w], in_=acc[:, j, :w])
            nc.scalar.dma_start(out=out_d, in_=out_sb)

    nc.compile()
    return nc


def _get_nc():
    if "nc" not in _NC_CACHE:
        _NC_CACHE["nc"] = _build_nc()
    return _NC_CACHE["nc"]


def _prep_in_maps(self_attn, self_delta, emb_table, value_w):
    self_attn = np.asarray(self_attn, dtype=np.float32)
    self_delta = np.asarray(self_delta, dtype=np.float32)
    emb_table = np.asarray(emb_table, dtype=np.float32)
    value_w = np.asarray(value_w, dtype=np.float32)

    embT_full = np.ascontiguousarray(emb_table[1 : L + 1].T)  # [EMB, L]

    # column r = n*M + m of attnT holds attn[n, m, :]; zero-pad to NTILE*P
    attnT = np.zeros((EMB, NTILE * P), dtype=np.float32)
    attnT[:, :R] = self_attn.transpose(2, 0, 1).reshape(EMB, R)

    # vwoh[p, t*N + j] = vw[m(r)] * (n(r) == j),  r = t*P + p
    vwoh = np.zeros((P, NTILE * N), dtype=np.float32)
    for t in range(NTILE):
        for p in range(min(P, R - t * P)):
            r = t * P + p
            vwoh[p, t * N + (r // M)] = value_w[r % M]

    in_maps = []
    for c in range(NCORES):
        lo = c * LSH
        raw_c = np.ascontiguousarray(
            self_delta[:, :, lo : lo + LSH, :]
        ).reshape(R, LSH * D)
        in_maps.append(
            {
                "raw": raw_c,
                "embT": np.ascontiguousarray(embT_full[:, lo : lo + LSH]),
                "attnT": attnT,
                "vwoh": vwoh,
            }
        )
    return in_maps


def _run(inputs, **spmd_kwargs):
    in_maps = _prep_in_maps(
        inputs["self_attn"], inputs["self_delta"], inputs["emb_table"], inputs["value_w"]
    )
    res = run_bass_kernel_spmd(
        _get_nc(), in_maps, core_ids=list(range(NCORES)), **spmd_kwargs
    )
    out = np.concatenate([r["out"] for r in res.results], axis=1)  # [N, L]
    return out, res


def kernel(**inputs) -> np.ndarray:
    out, _ = _run(inputs)
    return out
